# revision 36
# baseline (speedup 1.0000x reference)
"""Trainium2 Bass kernel for NewHyperLinear (hypernetwork linear layer).

Reference computation:
    params  = noise @ hyper_W.T + hyper_b            # [B, IN*OUT + OUT]
    out     = einsum('bi,bio->bo', x, params_w) + params_b
    (+ same with prior_x / prior_W / prior_b)

Key algebraic restructuring (avoids materializing the 537MB params tensor):
    out[b,o] = S[b,o] + sum_n noise[b,n] * Q_n[b,o]
    Q_n[b,o] = sum_i x[b,i]*W4h[n,i,o] + sum_i px[b,i]*W4p[n,i,o]
    S[b,o]   = x @ Bh + px @ Bp + noise @ (Wbh+Wbp) + (hb_tail + pb_tail)

where W4 is hyper_W's weight part reshaped/transposed to [n, i, o] (host-side).
Q_n is a plain matmul per n (both W's accumulate into one PSUM bank); the
per-sample noise scale is a scalar_tensor_tensor with per-partition scalars
(batch on partitions).

Sharding over 8 cores: 4-way over OUT_F columns x 2-way over batch.
"""

import numpy as np

import concourse.bass as bass
import concourse.bacc as bacc
import concourse.mybir as mybir
import concourse.tile as tile
from concourse.bass_utils import run_bass_kernel_spmd

B, IN_F, OUT_F, NOISE = 512, 512, 512, 128
W_PART = IN_F * OUT_F  # 262144
PRIOR_SCALE = 1.0

OG, BG = 4, 2                 # o-groups x b-groups = 8 cores
O_SL = OUT_F // OG            # 128 output cols per core
B_SL = B // BG                # 256 batch rows per core
BC = B_SL // 128              # 2 batch chunks of 128 (PSUM partition limit)
IC = IN_F // 128              # 4 contraction chunks
NB = 4                        # noise dims per block (one PSUM bank, N=512)
NBLK = NOISE // NB            # 32 blocks

F16 = mybir.dt.float16
F32 = mybir.dt.float32

_NC_CACHE = None


def _build_bass():
    nc = bacc.Bacc("TRN2", debug=False)

    # Per-core inputs. Layouts chosen so every dram->sbuf DMA is a single
    # transfer with >=1KB contiguous runs per partition.
    # xts packs x/prior_x transposed chunks; smalls packs bmat/pbmat/wb/btail.
    xts = nc.dram_tensor("xts", [128, 2, IC, B_SL], F16, kind="ExternalInput")
    noiset = nc.dram_tensor("noiset", [NOISE, B_SL], F16, kind="ExternalInput")
    noise = nc.dram_tensor("noise", [128, BC, NOISE], F32, kind="ExternalInput")
    wh = nc.dram_tensor("wh", [128, IC, NOISE, O_SL], F16, kind="ExternalInput")
    wp = nc.dram_tensor("wp", [128, IC, NOISE, O_SL], F16, kind="ExternalInput")
    smalls = nc.dram_tensor(
        "smalls", [128, 2 * IC + 2, O_SL], F16, kind="ExternalInput"
    )
    out = nc.dram_tensor("out", [128, BC, O_SL], F32, kind="ExternalOutput")

    ts = bass.ts

    with tile.TileContext(nc) as tc:
        with (
            tc.tile_pool(name="const", bufs=1) as cpool,
            tc.tile_pool(name="wpool", bufs=4) as wpool,
            tc.tile_pool(name="accp", bufs=1) as apool,
            tc.tile_pool(name="qpsum", bufs=7, space="PSUM") as qpool,
            tc.tile_pool(name="spsum", bufs=1, space="PSUM") as spool,
        ):
            # Four parallel HWDGE streams (~8.4MB each): weight halves on
            # Sync/Scalar/GpSimd/Vector queues. Consts are issued first.
            # x slices first (small, needed by the first matmuls), then the
            # first weight block in interleaved halves for the fastest ramp.
            # Ramp order: x/px slices interleaved with the first weight
            # blocks so the first matmuls' dependencies arrive earliest.
            # wh stream on Sync HWDGE, wp stream on Scalar HWDGE.
            xts_sb = cpool.tile([128, 2, IC, B_SL], F16)
            nc.sync.dma_start(xts_sb[:, 0, :, 0:128], xts[:, 0, :, 0:128])
            nc.scalar.dma_start(xts_sb[:, 1, :, 0:128], xts[:, 1, :, 0:128])
            xt_sb = xts_sb[:, 0]
            pxt_sb = xts_sb[:, 1]
            noise_sb = cpool.tile([128, BC, NOISE], F32)
            nc.gpsimd.dma_start(noise_sb[:], noise[:])
            noiset_sb = cpool.tile([NOISE, B_SL], F16)
            nc.gpsimd.dma_start(noiset_sb[:], noiset[:])
            smalls_sb = cpool.tile([128, 2 * IC + 2, O_SL], F16)
            nc.gpsimd.dma_start(smalls_sb[:], smalls[:])
            ones_sb = cpool.tile([1, 128], F16)
            nc.vector.memset(ones_sb[:], 1.0)

            PREFETCH = 3
            wh_tiles, wp_tiles = {}, {}

            def fetch_blk(blk):
                n0 = blk * NB
                wh_sb = wpool.tile([128, IC, NB, O_SL], F16, tag="wh")
                wp_sb = wpool.tile([128, IC, NB, O_SL], F16, tag="wp")
                nc.sync.dma_start(wh_sb[:], wh[:, :, n0:n0 + NB, :])
                nc.scalar.dma_start(wp_sb[:], wp[:, :, n0:n0 + NB, :])
                wh_tiles[blk], wp_tiles[blk] = wh_sb, wp_sb

            fetch_blk(0)
            nc.sync.dma_start(xts_sb[:, 0, :, 128:], xts[:, 0, :, 128:])
            nc.scalar.dma_start(xts_sb[:, 1, :, 128:], xts[:, 1, :, 128:])
            for blk in range(1, PREFETCH):
                fetch_blk(blk)

            acc = apool.tile([128, BC, O_SL], F32)

            # Touch noise_sb on DVE once so the per-n scalar_tensor_tensor
            # ops don't need their own DMA-wait (the S2S2D2_STT instruction
            # has a single sync-wait slot).
            scratch = cpool.tile([128, 1], F32)
            nc.vector.tensor_copy(scratch[:], noise_sb[:, 0, 0:1])

            # Warmup: full-size dummy matmuls (K=128, from a memset tile)
            # keep the PE busy through the DMA ramp so HAM un-throttles
            # (1.2->2.4GHz) before the real stream starts. Results are
            # garbage, overwritten by the secondary phase (start=True).
            warmsrc = cpool.tile([128, 128], F16)
            nc.vector.memset(warmsrc[:], 0.0)
            warm = spool.tile([128, BC, O_SL], F32, tag="sp")
            for _ in range(96):
                nc.tensor.matmul(warm[:, 0], warmsrc[:], warmsrc[:],
                                 start=True, stop=True)

            # Secondary terms: S = x@Bh + px@Bp + noise@Wb + 1x1@btail.
            # Emitted after blk 0 (PE fills DMA-ramp gaps); added to acc last.
            sp_tiles = {}

            def emit_secondary():
                spt = spool.tile([128, BC, O_SL], F32, tag="sp")
                for bc in range(BC):
                    sp = spt[:, bc]
                    for ic in range(IC):
                        nc.tensor.matmul(
                            sp[:], xt_sb[:, ic, ts(bc, 128)],
                            smalls_sb[:, ic, :],
                            start=(ic == 0), stop=False,
                        )
                    for ic in range(IC):
                        nc.tensor.matmul(
                            sp[:], pxt_sb[:, ic, ts(bc, 128)],
                            smalls_sb[:, IC + ic, :],
                            start=False, stop=False,
                        )
                    nc.tensor.matmul(
                        sp[:], noiset_sb[:, ts(bc, 128)],
                        smalls_sb[:, 2 * IC, :],
                        start=False, stop=False,
                    )
                    nc.tensor.matmul(
                        sp[:], ones_sb[:], smalls_sb[0:1, 2 * IC + 1, :],
                        start=False, stop=True,
                    )
                    sp_tiles[bc] = sp

            # Main loop: Q_n = x.T.T@W4h[n] + px.T.T@W4p[n]; acc += noise[:,n]*Q_n
            # blk 0 initializes acc via plain tensor_scalar (no read of acc),
            # so nothing upstream gates the first drains.
            for blk in range(NBLK):
                n0 = blk * NB
                wh_sb, wp_sb = wh_tiles.pop(blk), wp_tiles.pop(blk)
                if blk + PREFETCH < NBLK:
                    fetch_blk(blk + PREFETCH)
                for bc in range(BC):
                    q = qpool.tile([128, NB, O_SL], F32)
                    if True:
                        for ic in range(IC):
                            nc.tensor.matmul(
                                q[:], xt_sb[:, ic, ts(bc, 128)],
                                wh_sb[:, ic, :, :],
                                start=(ic == 0), stop=False,
                            )
                        for ic in range(IC):
                            nc.tensor.matmul(
                                q[:], pxt_sb[:, ic, ts(bc, 128)],
                                wp_sb[:, ic, :, :],
                                start=False, stop=(ic == IC - 1),
                            )
                        for j in range(NB):
                            if blk == 0 and j == 0:
                                nc.vector.tensor_scalar_mul(
                                    acc[:, bc, :], q[:, j, :],
                                    noise_sb[:, bc, n0:n0 + 1],
                                )
                            else:
                                nc.vector.scalar_tensor_tensor(
                                    acc[:, bc, :],
                                    q[:, j, :],
                                    noise_sb[:, bc, n0 + j:n0 + j + 1],
                                    acc[:, bc, :],
                                    mybir.AluOpType.mult,
                                    mybir.AluOpType.add,
                                )
                if blk == 0:
                    emit_secondary()
                elif blk == 1:
                    # Fold the secondary term into the accumulation chain
                    # early so it's off the tail critical path.
                    for bc in range(BC):
                        nc.vector.tensor_add(
                            acc[:, bc, :], acc[:, bc, :], sp_tiles[bc][:])

            nc.sync.dma_start(out[:], acc[:])

    nc.compile()
    return nc


def get_nc():
    global _NC_CACHE
    if _NC_CACHE is None:
        _NC_CACHE = _build_bass()
    return _NC_CACHE


def _prep_in_maps(x, prior_x, hyper_noise, hyper_W, hyper_b, prior_W, prior_b):
    f16, f32 = np.float16, np.float32
    x = np.asarray(x, f32)
    prior_x = np.asarray(prior_x, f32)
    hyper_noise = np.asarray(hyper_noise, f32)
    hyper_W = np.asarray(hyper_W, f32)
    hyper_b = np.asarray(hyper_b, f32)
    prior_W = np.asarray(prior_W, f32)
    prior_b = np.asarray(prior_b, f32)
    if PRIOR_SCALE != 1.0:
        prior_W = prior_W * PRIOR_SCALE
        prior_b = prior_b * PRIOR_SCALE

    W3h = hyper_W[:W_PART].reshape(IN_F, OUT_F, NOISE)
    W3p = prior_W[:W_PART].reshape(IN_F, OUT_F, NOISE)
    wbT = (hyper_W[W_PART:] + prior_W[W_PART:]).T          # [NOISE, OUT_F]
    bmat_full = hyper_b[:W_PART].reshape(IN_F, OUT_F)
    pbmat_full = prior_b[:W_PART].reshape(IN_F, OUT_F)
    btail_full = (hyper_b[W_PART:] + prior_b[W_PART:]).reshape(1, OUT_F)

    # per o-group arrays
    wh_c, wp_c, smalls_c = [], [], []
    for og in range(OG):
        osl = slice(og * O_SL, (og + 1) * O_SL)
        wh_c.append(
            W3h[:, osl, :].reshape(IC, 128, O_SL, NOISE)
            .transpose(1, 0, 3, 2).astype(f16)
        )
        wp_c.append(
            W3p[:, osl, :].reshape(IC, 128, O_SL, NOISE)
            .transpose(1, 0, 3, 2).astype(f16)
        )
        smalls = np.zeros((128, 2 * IC + 2, O_SL), f16)
        smalls[:, :IC, :] = (
            bmat_full[:, osl].reshape(IC, 128, O_SL).transpose(1, 0, 2)
        )
        smalls[:, IC:2 * IC, :] = (
            pbmat_full[:, osl].reshape(IC, 128, O_SL).transpose(1, 0, 2)
        )
        smalls[:, 2 * IC, :] = wbT[:, osl]
        smalls[0, 2 * IC + 1, :] = btail_full[0, osl]
        smalls_c.append(smalls)

    # per b-group arrays
    xts_c, noiset_c, noise_c = [], [], []
    for bg in range(BG):
        bsl = slice(bg * B_SL, (bg + 1) * B_SL)
        xts = np.stack([
            x[bsl].T.reshape(IC, 128, B_SL),
            prior_x[bsl].T.reshape(IC, 128, B_SL),
        ])  # [2, ic, ii, b]
        xts_c.append(xts.transpose(2, 0, 1, 3).astype(f16))
        noiset_c.append(np.ascontiguousarray(hyper_noise[bsl].T).astype(f16))
        noise_c.append(
            hyper_noise[bsl].reshape(BC, 128, NOISE).transpose(1, 0, 2)
            .astype(f32)
        )

    in_maps = []
    for cid in range(OG * BG):
        og, bg = cid % OG, cid // OG
        in_maps.append({
            "xts": xts_c[bg],
            "noiset": noiset_c[bg],
            "noise": noise_c[bg],
            "wh": wh_c[og],
            "wp": wp_c[og],
            "smalls": smalls_c[og],
        })
    return in_maps


def run(trace=False, **inputs):
    """Run the kernel; returns (full_output, BassKernelResults)."""
    nc = get_nc()
    in_maps = _prep_in_maps(**inputs)
    res = run_bass_kernel_spmd(
        nc, in_maps, core_ids=list(range(OG * BG)), trace=trace,
    )
    full = np.empty((B, OUT_F), np.float32)
    for cid in range(OG * BG):
        og, bg = cid % OG, cid // OG
        shard = (
            res.results[cid]["out"].transpose(1, 0, 2).reshape(B_SL, O_SL)
        )
        full[bg * B_SL:(bg + 1) * B_SL, og * O_SL:(og + 1) * O_SL] = shard
    return full, res


def kernel(**inputs):
    return run(trace=False, **inputs)[0]


# revision 38
# speedup vs baseline: 1.0124x; 1.0124x over previous
"""Trainium2 Bass kernel for NewHyperLinear (hypernetwork linear layer).

Reference computation:
    params  = noise @ hyper_W.T + hyper_b            # [B, IN*OUT + OUT]
    out     = einsum('bi,bio->bo', x, params_w) + params_b
    (+ same with prior_x / prior_W / prior_b)

Key algebraic restructuring (avoids materializing the 537MB params tensor):
    out[b,o] = S[b,o] + sum_n noise[b,n] * Q_n[b,o]
    Q_n[b,o] = sum_i x[b,i]*W4h[n,i,o] + sum_i px[b,i]*W4p[n,i,o]
    S[b,o]   = x @ Bh + px @ Bp + noise @ (Wbh+Wbp) + (hb_tail + pb_tail)

where W4 is hyper_W's weight part reshaped/transposed to [n, i, o] (host-side).
Q_n is a plain matmul per n (both W's accumulate into one PSUM bank); the
per-sample noise scale is a scalar_tensor_tensor with per-partition scalars
(batch on partitions).

Sharding over 8 cores: 4-way over OUT_F columns x 2-way over batch.
"""

import numpy as np

import concourse.bass as bass
import concourse.bacc as bacc
import concourse.mybir as mybir
import concourse.tile as tile
from concourse.bass_utils import run_bass_kernel_spmd

B, IN_F, OUT_F, NOISE = 512, 512, 512, 128
W_PART = IN_F * OUT_F  # 262144
PRIOR_SCALE = 1.0

OG, BG = 4, 2                 # o-groups x b-groups = 8 cores
O_SL = OUT_F // OG            # 128 output cols per core
B_SL = B // BG                # 256 batch rows per core
BC = B_SL // 128              # 2 batch chunks of 128 (PSUM partition limit)
IC = IN_F // 128              # 4 contraction chunks
NB = 4                        # noise dims per block (one PSUM bank, N=512)
NBLK = NOISE // NB            # 32 blocks

F16 = mybir.dt.float16
F32 = mybir.dt.float32

_NC_CACHE = None


def _build_bass():
    nc = bacc.Bacc("TRN2", debug=False)

    # Per-core inputs. Layouts chosen so every dram->sbuf DMA is a single
    # transfer with >=1KB contiguous runs per partition.
    # xts packs x/prior_x transposed chunks; smalls packs bmat/pbmat/wb/btail.
    xts = nc.dram_tensor("xts", [128, 2, IC, B_SL], F16, kind="ExternalInput")
    noiset = nc.dram_tensor("noiset", [NOISE, B_SL], F16, kind="ExternalInput")
    noise = nc.dram_tensor("noise", [128, BC, NOISE], F32, kind="ExternalInput")
    wh = nc.dram_tensor("wh", [128, IC, NOISE, O_SL], F16, kind="ExternalInput")
    wp = nc.dram_tensor("wp", [128, IC, NOISE, O_SL], F16, kind="ExternalInput")
    smalls = nc.dram_tensor(
        "smalls", [128, 2 * IC + 2, O_SL], F16, kind="ExternalInput"
    )
    out = nc.dram_tensor("out", [128, BC, O_SL], F32, kind="ExternalOutput")

    ts = bass.ts

    with tile.TileContext(nc) as tc:
        with (
            tc.tile_pool(name="const", bufs=1) as cpool,
            tc.tile_pool(name="wpool", bufs=4) as wpool,
            tc.tile_pool(name="accp", bufs=1) as apool,
            tc.tile_pool(name="qpsum", bufs=7, space="PSUM") as qpool,
            tc.tile_pool(name="spsum", bufs=1, space="PSUM") as spool,
        ):
            # Four parallel HWDGE streams (~8.4MB each): weight halves on
            # Sync/Scalar/GpSimd/Vector queues. Consts are issued first.
            # x slices first (small, needed by the first matmuls), then the
            # first weight block in interleaved halves for the fastest ramp.
            # Ramp order: x/px slices interleaved with the first weight
            # blocks so the first matmuls' dependencies arrive earliest.
            # wh stream on Sync HWDGE, wp stream on Scalar HWDGE.
            xts_sb = cpool.tile([128, 2, IC, B_SL], F16)
            nc.sync.dma_start(xts_sb[:, 0, :, 0:128], xts[:, 0, :, 0:128])
            nc.scalar.dma_start(xts_sb[:, 1, :, 0:128], xts[:, 1, :, 0:128])
            xt_sb = xts_sb[:, 0]
            pxt_sb = xts_sb[:, 1]
            noise_sb = cpool.tile([128, BC, NOISE], F32)
            nc.gpsimd.dma_start(noise_sb[:], noise[:])
            noiset_sb = cpool.tile([NOISE, B_SL], F16)
            nc.gpsimd.dma_start(noiset_sb[:], noiset[:])
            smalls_sb = cpool.tile([128, 2 * IC + 2, O_SL], F16)
            nc.gpsimd.dma_start(smalls_sb[:], smalls[:])
            ones_sb = cpool.tile([1, 128], F16)
            nc.vector.memset(ones_sb[:], 1.0)

            PREFETCH = 3
            wh_tiles, wp_tiles = {}, {}

            def fetch_blk(blk):
                n0 = blk * NB
                wh_sb = wpool.tile([128, IC, NB, O_SL], F16, tag="wh")
                wp_sb = wpool.tile([128, IC, NB, O_SL], F16, tag="wp")
                nc.sync.dma_start(wh_sb[:], wh[:, :, n0:n0 + NB, :])
                nc.scalar.dma_start(wp_sb[:], wp[:, :, n0:n0 + NB, :])
                wh_tiles[blk], wp_tiles[blk] = wh_sb, wp_sb

            fetch_blk(0)
            nc.sync.dma_start(xts_sb[:, 0, :, 128:], xts[:, 0, :, 128:])
            nc.scalar.dma_start(xts_sb[:, 1, :, 128:], xts[:, 1, :, 128:])
            for blk in range(1, PREFETCH):
                fetch_blk(blk)

            acc = apool.tile([128, BC, O_SL], F32)

            # Touch noise_sb on DVE once so the per-n scalar_tensor_tensor
            # ops don't need their own DMA-wait (the S2S2D2_STT instruction
            # has a single sync-wait slot).
            scratch = cpool.tile([128, 1], F32)
            nc.vector.tensor_copy(scratch[:], noise_sb[:, 0, 0:1])

            # Warmup: full-size dummy matmuls (K=128, from a memset tile)
            # keep the PE busy through the DMA ramp so HAM un-throttles
            # (1.2->2.4GHz) before the real stream starts. Results are
            # garbage, overwritten by the secondary phase (start=True).
            warmsrc = cpool.tile([128, 128], F16)
            nc.vector.memset(warmsrc[:], 0.0)
            warm = spool.tile([128, BC, O_SL], F32, tag="sp")
            for _ in range(96):
                nc.tensor.matmul(warm[:, 0], warmsrc[:], warmsrc[:],
                                 start=True, stop=True)

            # Secondary terms: S = x@Bh + px@Bp + noise@Wb + 1x1@btail.
            # Emitted after blk 0 (PE fills DMA-ramp gaps); added to acc last.
            sp_tiles = {}

            def emit_secondary():
                spt = spool.tile([128, BC, O_SL], F32, tag="sp")
                for bc in range(BC):
                    sp = spt[:, bc]
                    for ic in range(IC):
                        nc.tensor.matmul(
                            sp[:], xt_sb[:, ic, ts(bc, 128)],
                            smalls_sb[:, ic, :],
                            start=(ic == 0), stop=False,
                        )
                    for ic in range(IC):
                        nc.tensor.matmul(
                            sp[:], pxt_sb[:, ic, ts(bc, 128)],
                            smalls_sb[:, IC + ic, :],
                            start=False, stop=False,
                        )
                    nc.tensor.matmul(
                        sp[:], noiset_sb[:, ts(bc, 128)],
                        smalls_sb[:, 2 * IC, :],
                        start=False, stop=False,
                    )
                    nc.tensor.matmul(
                        sp[:], ones_sb[:], smalls_sb[0:1, 2 * IC + 1, :],
                        start=False, stop=True,
                    )
                    sp_tiles[bc] = sp

            # Main loop: Q_n = x.T.T@W4h[n] + px.T.T@W4p[n]; acc += noise[:,n]*Q_n
            # blk 0 initializes acc via plain tensor_scalar (no read of acc),
            # so nothing upstream gates the first drains.
            for blk in range(NBLK):
                n0 = blk * NB
                wh_sb, wp_sb = wh_tiles.pop(blk), wp_tiles.pop(blk)
                if blk + PREFETCH < NBLK:
                    fetch_blk(blk + PREFETCH)
                for bc in range(BC):
                    q = qpool.tile([128, NB, O_SL], F32)
                    if True:
                        for ic in range(IC):
                            nc.tensor.matmul(
                                q[:], xt_sb[:, ic, ts(bc, 128)],
                                wh_sb[:, ic, :, :],
                                start=(ic == 0), stop=False,
                            )
                        for ic in range(IC):
                            nc.tensor.matmul(
                                q[:], pxt_sb[:, ic, ts(bc, 128)],
                                wp_sb[:, ic, :, :],
                                start=False, stop=(ic == IC - 1),
                            )
                        for j in range(NB):
                            if blk == 0 and j == 0:
                                nc.vector.tensor_scalar_mul(
                                    acc[:, bc, :], q[:, j, :],
                                    noise_sb[:, bc, n0:n0 + 1],
                                )
                            else:
                                nc.vector.scalar_tensor_tensor(
                                    acc[:, bc, :],
                                    q[:, j, :],
                                    noise_sb[:, bc, n0 + j:n0 + j + 1],
                                    acc[:, bc, :],
                                    mybir.AluOpType.mult,
                                    mybir.AluOpType.add,
                                )
                if blk == 0:
                    emit_secondary()
                elif blk == 1:
                    # Fold the secondary term into the accumulation chain
                    # early so it's off the tail critical path.
                    for bc in range(BC):
                        nc.vector.tensor_add(
                            acc[:, bc, :], acc[:, bc, :], sp_tiles[bc][:])

            nc.sync.dma_start(out[:, 0], acc[:, 0])
            nc.sync.dma_start(out[:, 1], acc[:, 1])

    nc.compile()
    return nc


def get_nc():
    global _NC_CACHE
    if _NC_CACHE is None:
        _NC_CACHE = _build_bass()
    return _NC_CACHE


def _prep_in_maps(x, prior_x, hyper_noise, hyper_W, hyper_b, prior_W, prior_b):
    f16, f32 = np.float16, np.float32
    x = np.asarray(x, f32)
    prior_x = np.asarray(prior_x, f32)
    hyper_noise = np.asarray(hyper_noise, f32)
    hyper_W = np.asarray(hyper_W, f32)
    hyper_b = np.asarray(hyper_b, f32)
    prior_W = np.asarray(prior_W, f32)
    prior_b = np.asarray(prior_b, f32)
    if PRIOR_SCALE != 1.0:
        prior_W = prior_W * PRIOR_SCALE
        prior_b = prior_b * PRIOR_SCALE

    W3h = hyper_W[:W_PART].reshape(IN_F, OUT_F, NOISE)
    W3p = prior_W[:W_PART].reshape(IN_F, OUT_F, NOISE)
    wbT = (hyper_W[W_PART:] + prior_W[W_PART:]).T          # [NOISE, OUT_F]
    bmat_full = hyper_b[:W_PART].reshape(IN_F, OUT_F)
    pbmat_full = prior_b[:W_PART].reshape(IN_F, OUT_F)
    btail_full = (hyper_b[W_PART:] + prior_b[W_PART:]).reshape(1, OUT_F)

    # per o-group arrays
    wh_c, wp_c, smalls_c = [], [], []
    for og in range(OG):
        osl = slice(og * O_SL, (og + 1) * O_SL)
        wh_c.append(
            W3h[:, osl, :].reshape(IC, 128, O_SL, NOISE)
            .transpose(1, 0, 3, 2).astype(f16)
        )
        wp_c.append(
            W3p[:, osl, :].reshape(IC, 128, O_SL, NOISE)
            .transpose(1, 0, 3, 2).astype(f16)
        )
        smalls = np.zeros((128, 2 * IC + 2, O_SL), f16)
        smalls[:, :IC, :] = (
            bmat_full[:, osl].reshape(IC, 128, O_SL).transpose(1, 0, 2)
        )
        smalls[:, IC:2 * IC, :] = (
            pbmat_full[:, osl].reshape(IC, 128, O_SL).transpose(1, 0, 2)
        )
        smalls[:, 2 * IC, :] = wbT[:, osl]
        smalls[0, 2 * IC + 1, :] = btail_full[0, osl]
        smalls_c.append(smalls)

    # per b-group arrays
    xts_c, noiset_c, noise_c = [], [], []
    for bg in range(BG):
        bsl = slice(bg * B_SL, (bg + 1) * B_SL)
        xts = np.stack([
            x[bsl].T.reshape(IC, 128, B_SL),
            prior_x[bsl].T.reshape(IC, 128, B_SL),
        ])  # [2, ic, ii, b]
        xts_c.append(xts.transpose(2, 0, 1, 3).astype(f16))
        noiset_c.append(np.ascontiguousarray(hyper_noise[bsl].T).astype(f16))
        noise_c.append(
            hyper_noise[bsl].reshape(BC, 128, NOISE).transpose(1, 0, 2)
            .astype(f32)
        )

    in_maps = []
    for cid in range(OG * BG):
        og, bg = cid % OG, cid // OG
        in_maps.append({
            "xts": xts_c[bg],
            "noiset": noiset_c[bg],
            "noise": noise_c[bg],
            "wh": wh_c[og],
            "wp": wp_c[og],
            "smalls": smalls_c[og],
        })
    return in_maps


def run(trace=False, trace_cores=None, **inputs):
    """Run the kernel; returns (full_output, BassKernelResults)."""
    nc = get_nc()
    in_maps = _prep_in_maps(**inputs)
    res = run_bass_kernel_spmd(
        nc, in_maps, core_ids=list(range(OG * BG)), trace=trace,
        trace_cores=trace_cores,
    )
    full = np.empty((B, OUT_F), np.float32)
    for cid in range(OG * BG):
        og, bg = cid % OG, cid // OG
        shard = (
            res.results[cid]["out"].transpose(1, 0, 2).reshape(B_SL, O_SL)
        )
        full[bg * B_SL:(bg + 1) * B_SL, og * O_SL:(og + 1) * O_SL] = shard
    return full, res


def kernel(**inputs):
    return run(trace=False, **inputs)[0]
